# revision 45
# baseline (speedup 1.0000x reference)
"""Trainium2 Bass kernel for causal multi-head attention (B=2, S=2048, D=1024, 16 heads x 64).

Sharding: 8 cores = 2 batches x 4 head-groups (tensor parallel over heads).
Each core computes attention for its 4 heads; the 4 cores of a batch
AllGather normalized head outputs per q-quarter, and each core applies the
full W_O projection to a 128-row slice of EVERY quarter (so the projection
overlaps the remaining attention work and only the last, smallest quarter's
collective is exposed at the tail). Host concatenates the 128-row chunks.

Attention is flash-style with transposed scores:
  sT[k, q] = K Q^T  (k on partitions), pattern = exp(sT/8) on ACT,
  AV uses stationary [v | 1] so PSUM row 64 accumulates the softmax
  denominator for free. Causality at tile granularity: upper-triangular
  tiles are skipped and diagonal tiles are column-trimmed + masked with a
  single 128x128 triangle.
Normalization: reciprocal_approx_fast on the denominator row, broadcast
across 64 partitions on the (otherwise idle) GpSimd engine, then one
vector multiply per head.
"""

import os
import sys

sys.path.insert(0, "/opt/trn_rl_repo")

import numpy as np

# ---- problem constants (hardcoded; kernel.py must be self-contained) ----
B = 2
S = 2048
D = 1024
N_HEADS = 16
DH = 64                 # head dim
NCORES = 8
NH_CORE = N_HEADS // 4  # 4 heads per core (4-way TP x 2-way batch DP)
SCALE = 1.0 / 8.0       # 1/sqrt(64)

P = 128                 # partitions
DC = D // P             # 8 contraction chunks for the projections
KC = S // P             # 16 key chunks
QT = 512                # q tile width (free dim) for scores / AV
NQT = S // QT           # 4 q tiles
NT = 512                # moving-operand tile for projections / out-proj
GRP = 4                 # cores per batch group

_CACHE = {}


def _build():
    import concourse.bass as bass
    import concourse.tile as tile
    from concourse import bacc, mybir

    f32 = mybir.dt.float32
    F16 = mybir.dt.float16

    nc = bacc.Bacc(
        "TRN2",
        target_bir_lowering=False,
        debug=False,
        enable_asserts=False,
        num_devices=NCORES,
    )

    xt_d = nc.dram_tensor("xt", [D, S], F16, kind="ExternalInput").ap()
    wqt_d = nc.dram_tensor("wqt", [D, NH_CORE * DH], F16, kind="ExternalInput").ap()
    wkt_d = nc.dram_tensor("wkt", [D, NH_CORE * DH], F16, kind="ExternalInput").ap()
    wvt_d = nc.dram_tensor("wvt", [D, NH_CORE * DH], F16, kind="ExternalInput").ap()
    wof_d = nc.dram_tensor("wof", [N_HEADS * DH, D], F16, kind="ExternalInput").ap()
    msk_d = nc.dram_tensor("msk", [P, P], F16, kind="ExternalInput").ap()
    # output: one 128-row chunk per q-quarter (this core's slice of each quarter)
    out_d = nc.dram_tensor("out", [NQT * P, D], f32, kind="ExternalOutput").ap()
    dbg_mode = int(os.environ.get("KERNEL_DEBUG", "0"))
    dbg_d = None
    if dbg_mode:
        dbg_d = nc.dram_tensor(
            "dbg", [NQT * (2 * P + GRP * 2 * P), QT], F16, kind="ExternalOutput"
        ).ap()

    Exp = mybir.ActivationFunctionType.Exp

    with tile.TileContext(nc) as tc:
        with (
            tc.tile_pool(name="const", bufs=1) as const,
            tc.tile_pool(name="work", bufs=2) as work,
            tc.tile_pool(name="ps", bufs=2, space="PSUM") as ps_pool,
            tc.tile_pool(name="dram", bufs=1, space="DRAM") as dram,
        ):
            late_cm = tc.tile_pool(name="late", bufs=1)
            late = late_cm.__enter__()
            xt_pool_cm = tc.tile_pool(name="xtp", bufs=1)
            xt_pool = xt_pool_cm.__enter__()
            # warm up the CC rings during the input-DMA window: the first
            # collective on cold rings costs ~20us extra
            wrm_in = dram.tile([2, DH], F16)
            wrm_out = dram.tile([NCORES * 2, DH], F16, addr_space="Shared")
            nc.gpsimd.collective_compute(
                "AllGather",
                mybir.AluOpType.bypass,
                replica_groups=[[0, 1, 2, 3, 4, 5, 6, 7]],
                ins=[wrm_in[:].opt()],
                outs=[wrm_out[:].opt()],
            )
            # ---------------- input DMAs ----------------
            wq_sb = xt_pool.tile([P, DC, NH_CORE * DH], F16)
            nc.sync.dma_start(wq_sb[:], wqt_d.rearrange("(c p) n -> p c n", p=P))
            wk_sb = xt_pool.tile([P, DC, NH_CORE * DH], F16)
            nc.sync.dma_start(wk_sb[:], wkt_d.rearrange("(c p) n -> p c n", p=P))

            # residual^T as separate tiles per (d-chunk, s-half) so the first
            # projections depend only on the first half's DMAs
            xt_r = xt_d.rearrange("(c p) s -> p c s", p=P)
            SH = S // 2
            xt_sb = [
                [xt_pool.tile([P, SH], F16, name=f"xt{dc}_{h}") for h in range(2)]
                for dc in range(DC)
            ]
            for dc in range(DC):
                nc.sync.dma_start(xt_sb[dc][0][:], xt_r[:, dc, 0:SH])
            wv_sb = xt_pool.tile([P, DC, NH_CORE * DH], F16)
            nc.sync.dma_start(wv_sb[:], wvt_d.rearrange("(c p) n -> p c n", p=P))
            for dc in range(DC):
                nc.sync.dma_start(xt_sb[dc][1][:], xt_r[:, dc, SH:S])
            tri_sb = const.tile([P, P], F16)
            nc.sync.dma_start(tri_sb[:], msk_d)
            # full W_O: needed from the first out_proj (~150us in); issue last
            # in the initial DMA window so it never contends with AllGathers
            wo_sb = const.tile([P, DC, D], F16)
            nc.sync.dma_start(wo_sb[:], wof_d.rearrange("(c p) d -> p c d", p=P))

            def xt_cols(dc, lo, hi):
                """xt_sb slice for columns [lo, hi) — must stay in one half."""
                h = lo // SH
                assert hi <= (h + 1) * SH
                return xt_sb[dc][h][:, lo - h * SH : hi - h * SH]

            # ---------------- QKV projections ----------------
            qT = [const.tile([P, S], F16, name=f"qT{i}") for i in range(2)]
            kT = [const.tile([P, S], F16, name=f"kT{i}") for i in range(2)]

            def qk_proj(ntile):
                for pr in range(2):
                    for w_sb, dst in ((wq_sb, qT[pr]), (wk_sb, kT[pr])):
                        pp = ps_pool.tile(
                            [P, NT], f32, name="pp", tag=f"op{ntile % 2}", bufs=1
                        )
                        for dc in range(DC):
                            nc.tensor.matmul(
                                pp[:],
                                w_sb[:, dc, pr * P : (pr + 1) * P],
                                xt_cols(dc, ntile * NT, (ntile + 1) * NT),
                                start=(dc == 0),
                                stop=(dc == DC - 1),
                            )
                        nc.scalar.copy(dst[:, ntile * NT : (ntile + 1) * NT], pp[:])
            for nt in range(2):
                qk_proj(nt)

            # v in natural [k, h] layout with an appended ones column: the AV
            # matmul accumulates the softmax denominator on PSUM partition 64
            v_aug = [const.tile([P, KC, DH + 1], F16, name=f"vaug{h}") for h in range(NH_CORE)]
            ones_f32 = const.tile([P, DH], f32)
            nc.vector.memset(ones_f32[:], 1.0)
            # f16 ones row AT partition 64 (stationary/moving of the
            # denominator-broadcast matmul must share base partition 64)
            ones_f16 = const.tile([DH + 1, DH], F16)
            nc.scalar.copy(ones_f16[DH : DH + 1, :], ones_f32[0:1, :])
            for h in range(NH_CORE):
                nc.scalar.copy(v_aug[h][:, :, DH : DH + 1], ones_f32[:, 0:KC, None])
            def v_proj(pc):
                vp = ps_pool.tile(
                    [P, NH_CORE * DH], f32, name="vp", tag=f"op{pc % 2}", bufs=1
                )
                for dc in range(DC):
                    nc.tensor.matmul(
                        vp[:],
                        xt_cols(dc, pc * P, (pc + 1) * P),
                        wv_sb[:, dc, :],
                        start=(dc == 0),
                        stop=(dc == DC - 1),
                    )
                for h in range(NH_CORE):
                    nc.vector.tensor_copy(
                        v_aug[h][:, pc, 0:DH],
                        vp[:, h * DH : (h + 1) * DH],
                    )

            # Per-quarter collective state: cc_in[g] rows [pr*128+h2*64+dh]
            # = normalized attn_outT of head (pr*2+h2) for q-quarter g.
            # AllGather over the 4-core batch group -> cc_out[g] rows
            # [cg*256 + pr*128 + h2*64] (group-core-major = W_O row order).
            cc_in = [dram.tile([2 * P, QT], F16, name=f"cci{g}") for g in range(NQT)]
            cc_out = [
                dram.tile(
                    [NCORES * 2 * P, QT], F16, name=f"cco{g}", addr_space="Shared"
                )
                for g in range(NQT)
            ]
            rep_groups = [[0, 1, 2, 3, 4, 5, 6, 7]]

            def norm_head(qtg, pr, h2):
                """normalize head (pr,h2) for quarter qtg and DMA to cc_in."""
                acc = attn_ps[pr][h2]
                # denominator row (PSUM p64) -> f16 SBUF, broadcast to 64
                # partitions at p0 via a tiny matmul (stationary = ones row),
                # approx-reciprocal straight off that PSUM block, multiply
                den = work.tile([DH + 1, QT], F16, name="den", tag=f"dn{pr}{h2}", bufs=1)
                nc.vector.tensor_copy(den[DH : DH + 1, :], acc[DH : DH + 1, :])
                rb_ps = ps_pool.tile([DH, QT], f32, name="rb", tag=f"s{h2}", bufs=1)
                nc.tensor.matmul(
                    rb_ps[:], ones_f16[DH : DH + 1, :], den[DH : DH + 1, :],
                    start=True, stop=True,
                )
                rb = work.tile([DH, QT], f32, name="rbs", tag=f"rb{pr}{h2}", bufs=1)
                nc.vector.reciprocal_approx_fast(rb[:], rb_ps[:])
                u_n = work.tile([DH, QT], F16, name="u_n", bufs=4)
                nc.vector.tensor_mul(u_n[:], acc[0:DH, :], rb[:])
                row = pr * P + h2 * DH
                nc.sync.dma_start(cc_in[qtg][row : row + DH, :], u_n[:])

            def gather(qtg):
                nc.gpsimd.collective_compute(
                    "AllGather",
                    mybir.AluOpType.bypass,
                    replica_groups=rep_groups,
                    ins=[cc_in[qtg][:].opt()],
                    outs=[cc_out[qtg][:].opt()],
                )

            # --------- per-quarter W_O projection on own 128-col slice ---------
            # this core's q columns within each quarter: [(pid%4)*128, +128)
            pid = nc.partition_id()
            coff = nc.snap(
                nc.s_assert_within(
                    (pid % GRP) * P, 0, QT - P, skip_runtime_assert=True
                )
            )
            aoff = nc.snap(
                nc.s_assert_within(
                    (pid - (pid % GRP)) * 2, 0, NCORES * 2 - DC,
                    skip_runtime_assert=True,
                )
            )

            def out_proj(qtg):
                attR = work.tile([P, DC, P], F16, name="attR", tag=f"aR{qtg % 2}", bufs=1)
                cc_r = cc_out[qtg][:].rearrange("(a p) q -> p a q", p=P)
                nc.sync.dma_start(
                    attR[:], cc_r[:, bass.ds(aoff, DC), bass.ds(coff, P)]
                )
                osb = work.tile([P, D], f32, name="osb", bufs=2)
                for dt_ in range(D // NT):
                    op = ps_pool.tile([P, NT], f32, name="op", tag=f"op{dt_ % 2}", bufs=1)
                    for c in range(DC):
                        nc.tensor.matmul(
                            op[:],
                            attR[:, c, :],
                            wo_sb[:, c, dt_ * NT : (dt_ + 1) * NT],
                            start=(c == 0),
                            stop=(c == DC - 1),
                        )
                    nc.scalar.copy(osb[:, dt_ * NT : (dt_ + 1) * NT], op[:])
                nc.sync.dma_start(out_d[qtg * P : (qtg + 1) * P, :], osb[:])

            # ---------------- attention ----------------
            for pc in range(DC):
                v_proj(pc)
            qt_order = [1, 2, 3, 0]  # dense-ish start, early AG pipeline, small tail
            for qi, qt in enumerate(qt_order):
                if qi >= 2:
                    # two-quarter lag, issued BEFORE this quarter so the attR
                    # DMA trigger isn't stuck behind this quarter's norm DMAs
                    # in the in-order sync queue
                    out_proj(qt_order[qi - 2])
                if qi == 1:
                    # rest of the projections, overlapping qt1's epilogue
                    for nt in range(2, S // NT):
                        qk_proj(nt)
                    for pc in range(DC, KC):
                        v_proj(pc)
                    xt_pool_cm.__exit__(None, None, None)  # frees xt + wq/wk/wv
                q_sl = slice(qt * QT, (qt + 1) * QT)
                nk = (qt + 1) * (QT // P)
                attn_ps = [
                    [
                        ps_pool.tile([DH + 1, QT], f32, name=f"attn{pr}{h2}", tag=f"a{pr}{h2}", bufs=1)
                        for h2 in range(2)
                    ]
                    for pr in range(2)
                ]
                for pr in range(2):
                    for kb in range(nk):
                        k_sl = slice(kb * P, (kb + 1) * P)
                        ri = kb - qt * (QT // P)  # >= 0 on diagonal tiles
                        r = max(ri, 0) * P        # first valid column in this q tile
                        c_sl = slice(qt * QT + r, (qt + 1) * QT)
                        s_ps = [
                            ps_pool.tile([P, QT], f32, name=f"s{h2}", tag=f"s{h2}", bufs=1)
                            for h2 in range(2)
                        ]
                        for h2 in range(2):
                            hb = h2 * DH
                            nc.tensor.matmul(
                                s_ps[h2][:, r:QT],
                                kT[pr][hb : hb + DH, k_sl],
                                qT[pr][hb : hb + DH, c_sl],
                                start=True,
                                stop=True,
                            )
                        for h2 in range(2):
                            h = pr * 2 + h2
                            pat = work.tile([P, QT], F16, name="pat", bufs=4)
                            nc.scalar.activation(
                                pat[:, r:QT], s_ps[h2][:, r:QT], Exp, scale=SCALE
                            )
                            if ri >= 0:
                                nc.vector.tensor_mul(
                                    pat[:, r : r + P], pat[:, r : r + P], tri_sb[:]
                                )
                            nc.tensor.matmul(
                                attn_ps[pr][h2][0 : DH + 1, r:QT],
                                v_aug[h][:, kb, :],
                                pat[:, r:QT],
                                start=(kb == 0),
                                stop=(kb == nk - 1),
                            )
                    for h2 in range(2):
                        norm_head(qt, pr, h2)
                gather(qt)
            out_proj(qt_order[-2])
            out_proj(qt_order[-1])
            if dbg_mode:
                CI, CO = 2 * P, GRP * 2 * P
                for g in range(NQT):
                    base = g * (CI + CO)
                    nc.sync.dma_start(dbg_d[base : base + CI, :], cc_in[g][:])
                    nc.sync.dma_start(
                        dbg_d[base + CI : base + CI + CO, :], cc_out[g][:]
                    )
            late_cm.__exit__(None, None, None)

    nc.compile()
    return nc


def _get_nc():
    if "nc" not in _CACHE:
        _CACHE["nc"] = _build()
    return _CACHE["nc"]


def _tri():
    k = np.arange(P)[:, None]
    q = np.arange(P)[None, :]
    return (q >= k).astype(np.float32)


def _ensure_ntff_hook():
    """Register the axon NTFF profile hook (missing antenv.axon_hooks shim)."""
    import types

    try:
        from antenv.axon_hooks import get_axon_ntff_profile_hook  # noqa: F401

        return
    except ImportError:
        pass
    import antenv

    if "/root/.axon_site" not in sys.path:
        sys.path.insert(0, "/root/.axon_site")
    from trn_agent_boot.trn_boot import _ntff_profile_via_ctypes

    hook = _ntff_profile_via_ctypes("/opt/axon/libaxon_pjrt.so")
    mod = types.ModuleType("antenv.axon_hooks")
    mod.get_axon_ntff_profile_hook = lambda: hook
    mod.set_axon_ntff_profile_hook = lambda h: None
    sys.modules["antenv.axon_hooks"] = mod
    antenv.axon_hooks = mod


def kernel(residual, W_Q, W_K, W_V, W_O):
    from concourse.bass_utils import run_bass_kernel_spmd

    if int(os.environ.get("KERNEL_TRACE", "0")):
        _ensure_ntff_hook()

    residual = np.ascontiguousarray(np.asarray(residual), np.float32)
    W_Q = np.ascontiguousarray(np.asarray(W_Q), np.float32)
    W_K = np.ascontiguousarray(np.asarray(W_K), np.float32)
    W_V = np.ascontiguousarray(np.asarray(W_V), np.float32)
    W_O = np.ascontiguousarray(np.asarray(W_O), np.float32)

    nc = _get_nc()
    tri = _tri()
    wof = np.ascontiguousarray(W_O.reshape(N_HEADS * DH, D).astype(np.float16))
    in_maps = []
    for c in range(NCORES):
        b, g = divmod(c, GRP)
        hs = slice(g * NH_CORE, (g + 1) * NH_CORE)
        in_maps.append(
            {
                "xt": np.ascontiguousarray(residual[b].T.astype(np.float16)),
                "wqt": np.ascontiguousarray(
                    W_Q[hs].transpose(2, 0, 1).reshape(D, NH_CORE * DH).astype(np.float16)
                ),
                "wkt": np.ascontiguousarray(
                    W_K[hs].transpose(2, 0, 1).reshape(D, NH_CORE * DH).astype(np.float16)
                ),
                "wvt": np.ascontiguousarray(
                    W_V[hs].transpose(2, 0, 1).reshape(D, NH_CORE * DH).astype(np.float16)
                ),
                "wof": wof,
                "msk": tri.astype(np.float16),
            }
        )

    res = run_bass_kernel_spmd(
        nc,
        in_maps,
        core_ids=list(range(NCORES)),
        trace=bool(int(os.environ.get("KERNEL_TRACE", "0"))),
        trace_cores=[0] if int(os.environ.get("KERNEL_TRACE", "0")) else None,
    )
    _CACHE["last_results"] = res

    # core (b, g) chunk qtg -> out[b, qtg*512 + g*128 : +128]
    out = np.empty((B, S, D), np.float32)
    for b in range(B):
        for g in range(GRP):
            chunks = res.results[b * GRP + g]["out"]
            for qtg in range(NQT):
                out[b, qtg * QT + g * P : qtg * QT + (g + 1) * P] = chunks[
                    qtg * P : (qtg + 1) * P
                ]
    return out
